# revision 4
# baseline (speedup 1.0000x reference)
"""MoH (Mixture-of-Heads) attention kernel for Trainium2, 8 NeuronCores.

Strategy: data-parallel over batch (32 batches -> 4 per core), weights
replicated, no collectives. Inside each core everything is computed with
bf16 matmuls (fp32 PSUM accumulation):

  - host pre-transposes q/k/v to [B, D, S] and casts to bf16, so the
    projections need no on-device transposes:
       qpT[d',s] = sum_d Wq[d,d'] qT[d,s]   (lhsT = Wq as stored)
       vp[s,d']  = sum_d vT[d,s] Wv[d,d']   (lhsT = vT)
  - heads live in the partition dim of qpT/kpT (64 rows each), so
    transposed scores ST[k,q] = kh @ qh^T come straight from matmuls of
    qpT/kpT slices; softmax runs without max-subtraction (scores are
    O(1)), masked blocks are skipped entirely, the causal diagonal gets
    an additive -1e9 mask tile, and the reference's "row 0 := 0" rule
    becomes "STexp[:,0] := 1" plus ones-matmuls for skipped k-blocks.
  - vp carries a ones-column so the attention matmul also produces the
    softmax denominator (row 64 of the [65, q] PSUM tile).
  - routing: gates = softmax(qpT^T @ Wg), hard top-2 of 12 via two
    reduce_max passes, mean over S via a ones-vector matmul; the
    per-(batch,head) routing scalar and 1/denominator are fused into the
    single DVE op that moves ctx^T from PSUM to SBUF.
  - out = ctxT^T @ Wo from the same partition-sliced ctxT tiles.
"""

import sys

_TRN_REPO = "/opt/trn_rl_repo"
if _TRN_REPO not in sys.path:
    sys.path.insert(0, _TRN_REPO)

import numpy as np
import ml_dtypes

B, S, D = 32, 512, 1024
H, DK = 16, 64
H_SH, K_SEL = 4, 2
H_DYN = H - H_SH
N_CORES = 8
B_LOC = B // N_CORES
SB = S // 128      # 4 s-blocks
DT = D // 128      # 8 d-tiles
NEG = -1e9

_CACHE = {}
PROFILE = False          # set by test harness to capture an NTFF trace
LAST = {}                # exec_time_ns / profile path from the last run


def _classify_mask(mask):
    """Host-side: derive block structure from the [S,S] 0/1 mask.

    Returns (qs, mixed, uniq_tiles) where
      qs[kb]    = first q (multiple of 128) kept for k-block kb, or None
      mixed[(qb,kb)] = index into uniq_tiles for blocks needing an
                  additive mask tile (maskT layout [k_local, q_local])
      uniq_tiles = list of [128,128] f32 additive tiles
    """
    m = mask.astype(bool)
    if not m[1:].any(axis=1).all():
        raise NotImplementedError(
            "a query row (>0) is fully masked; uniform-softmax fallback "
            "for fully-masked rows is not implemented"
        )
    qs = []
    mixed = {}
    uniq = []
    uniq_key = {}
    for kb in range(SB):
        first = None
        for qb in range(SB):
            blk = m[qb * 128:(qb + 1) * 128, kb * 128:(kb + 1) * 128]
            if blk.any():
                if first is None:
                    first = qb * 128
                if not blk.all():
                    add = np.where(blk.T, 0.0, np.float32(NEG)).astype(np.float32)
                    key = add.tobytes()
                    if key not in uniq_key:
                        uniq_key[key] = len(uniq)
                        uniq.append(add)
                    mixed[(qb, kb)] = uniq_key[key]
            elif first is not None:
                # fully-masked block inside the computed q range: the
                # scores are computed but must not contribute
                mixed[(qb, kb)] = -1  # sentinel: zero it after exp
        qs.append(first)
    return qs, mixed, uniq


def _build(mask_key, qs, mixed, uniq_n, b_loc=B_LOC, has_bvo=False):
    import concourse.bacc as bacc
    import concourse.tile as tile
    import concourse.mybir as mybir
    from contextlib import ExitStack

    f32 = mybir.dt.float32
    bf16 = mybir.dt.bfloat16
    AF = mybir.ActivationFunctionType
    ALU = mybir.AluOpType

    nc = bacc.Bacc(trn_type="TRN2", target_bir_lowering=False, debug=False)

    qT = nc.dram_tensor("qT", [b_loc, D, S], bf16, kind="ExternalInput").ap()
    kT = nc.dram_tensor("kT", [b_loc, D, S], bf16, kind="ExternalInput").ap()
    vT = nc.dram_tensor("vT", [b_loc, D, S], bf16, kind="ExternalInput").ap()
    wq = nc.dram_tensor("wq", [D, D], bf16, kind="ExternalInput").ap()
    wk = nc.dram_tensor("wk", [D, D], bf16, kind="ExternalInput").ap()
    wv = nc.dram_tensor("wv", [D, D], bf16, kind="ExternalInput").ap()
    wo = nc.dram_tensor("wo", [D, D], bf16, kind="ExternalInput").ap()
    wg = nc.dram_tensor("wg", [D, H_DYN], bf16, kind="ExternalInput").ap()
    bqt = nc.dram_tensor("bqt", [128, DT], f32, kind="ExternalInput").ap()
    bkt = nc.dram_tensor("bkt", [128, DT], f32, kind="ExternalInput").ap()
    if has_bvo:
        bvb = nc.dram_tensor("bvb", [1, D], f32, kind="ExternalInput").ap()
        bob = nc.dram_tensor("bob", [1, D], f32, kind="ExternalInput").ap()
    if uniq_n:
        maskt = nc.dram_tensor(
            "maskt", [uniq_n, 128, 128], f32, kind="ExternalInput"
        ).ap()
    out = nc.dram_tensor("out", [b_loc, S, D], f32, kind="ExternalOutput").ap()

    with tile.TileContext(nc) as tc, ExitStack() as ctx:
        const = ctx.enter_context(tc.tile_pool(name="const", bufs=1))
        act = ctx.enter_context(tc.tile_pool(name="act", bufs=2))
        small = ctx.enter_context(tc.tile_pool(name="small", bufs=2))
        psum = ctx.enter_context(tc.tile_pool(name="psum", bufs=1, space="PSUM"))

        # ---- constants -------------------------------------------------
        w_tiles = {}
        for wname, wap in (("wq", wq), ("wk", wk), ("wv", wv), ("wo", wo)):
            tl = []
            for d in range(DT):
                t = const.tile([128, D], bf16, name=f"{wname}{d}", tag=f"{wname}{d}")
                nc.sync.dma_start(t[:], wap[d * 128:(d + 1) * 128, :])
                tl.append(t)
            w_tiles[wname] = tl
        wg_tiles = []
        for d in range(DT):
            t = const.tile([128, H_DYN], bf16, name=f"wg{d}", tag=f"wg{d}")
            nc.sync.dma_start(t[:], wg[d * 128:(d + 1) * 128, :])
            wg_tiles.append(t)
        bq_sb = const.tile([128, DT], f32, name="bq_sb", tag="bq_sb")
        nc.sync.dma_start(bq_sb[:], bqt[:])
        bk_sb = const.tile([128, DT], f32, name="bk_sb", tag="bk_sb")
        nc.sync.dma_start(bk_sb[:], bkt[:])
        if has_bvo:
            bv_sb = const.tile([1, D], f32, name="bv_sb", tag="bv_sb")
            nc.sync.dma_start(bv_sb[:], bvb[:])
            bo_sb = const.tile([1, D], f32, name="bo_sb", tag="bo_sb")
            nc.sync.dma_start(bo_sb[:], bob[:])
            bvb_sb = const.tile([128, D], f32, name="bvb_sb", tag="bvb_sb")
            nc.gpsimd.partition_broadcast(bvb_sb[:], bv_sb[:])
            bob_sb = const.tile([128, D], f32, name="bob_sb", tag="bob_sb")
            nc.gpsimd.partition_broadcast(bob_sb[:], bo_sb[:])

        mask_tiles = []
        for u in range(uniq_n):
            t = const.tile([128, 128], f32, name=f"mask{u}", tag=f"mask{u}")
            nc.sync.dma_start(t[:], maskt[u])
            mask_tiles.append(t)

        ones_bf = const.tile([128, 1], bf16, name="ones_bf", tag="ones_bf")
        nc.vector.memset(ones_bf[:], 1.0)
        ones_f32 = const.tile([128, 1], f32, name="ones_f32", tag="ones_f32")
        nc.vector.memset(ones_f32[:], 1.0)

        # ---- per-batch pipeline ---------------------------------------
        for b in range(b_loc):
            # input tiles
            ins = {}
            for nm, ap in (("q", qT), ("k", kT), ("v", vT)):
                t = act.tile([128, DT, S], bf16, name=f"in_{nm}", tag=f"in_{nm}", bufs=1)
                for d in range(DT):
                    nc.sync.dma_start(t[:, d, :], ap[b, d * 128:(d + 1) * 128, :])
                ins[nm] = t

            # ---- projections ------------------------------------------
            qpT = act.tile([128, DT, S], bf16, name="qpT", tag="qpT")
            kpT = act.tile([128, DT, S], bf16, name="kpT", tag="kpT")
            for dst, src, wn, bias in (
                (qpT, ins["q"], "wq", bq_sb),
                (kpT, ins["k"], "wk", bk_sb),
            ):
                for t in range(DT):
                    ps = psum.tile([128, S], f32, name="mm_ps", tag="mm", bufs=3)
                    for d in range(DT):
                        nc.tensor.matmul(
                            ps[:],
                            w_tiles[wn][d][:, t * 128:(t + 1) * 128],
                            src[:, d, :],
                            start=(d == 0),
                            stop=(d == DT - 1),
                        )
                    nc.scalar.activation(
                        dst[:, t, :], ps[:], AF.Identity,
                        bias=bias[:, t:t + 1],
                    )

            # vp: [s, h, 65] with ones column (65th) for the denominator
            vp = act.tile([128, SB, H, DK + 1], bf16, name="vp", tag="vp")
            nc.vector.memset(vp[:, :, :, DK:DK + 1], 1.0)
            for sb in range(SB):
                for c in range(2):  # d' chunks of 512
                    ps = psum.tile([128, S], f32, name="mmv_ps", tag="mm", bufs=3)
                    for d in range(DT):
                        nc.tensor.matmul(
                            ps[:],
                            ins["v"][:, d, sb * 128:(sb + 1) * 128],
                            w_tiles["wv"][d][:, c * 512:(c + 1) * 512],
                            start=(d == 0),
                            stop=(d == DT - 1),
                        )
                    src = ps[:].rearrange("p (h e) -> p h e", e=DK)
                    dst = vp[:, sb, c * 8:(c + 1) * 8, 0:DK]
                    if has_bvo:
                        nc.vector.scalar_tensor_tensor(
                            dst, src, 1.0,
                            bvb_sb[:, c * 512:(c + 1) * 512].rearrange(
                                "p (h e) -> p h e", e=DK),
                            op0=ALU.mult, op1=ALU.add,
                        )
                    else:
                        nc.vector.tensor_copy(dst, src)

            # ---- routing gates ----------------------------------------
            ps_r = psum.tile([1, H_DYN], f32, name="ps_r", tag="rsum", bufs=1)
            for sb in range(SB):
                ps_g = psum.tile([128, H_DYN], f32, name="ps_g", tag="gat", bufs=2)
                for t in range(DT):
                    nc.tensor.matmul(
                        ps_g[:],
                        qpT[:, t, sb * 128:(sb + 1) * 128],
                        wg_tiles[t][:],
                        start=(t == 0),
                        stop=(t == DT - 1),
                    )
                gexp = small.tile([128, H_DYN], f32, name="gexp", tag="gexp")
                gsum = small.tile([128, 1], f32, name="gsum", tag="gsum")
                nc.scalar.activation(
                    gexp[:], ps_g[:], AF.Exp, accum_out=gsum[:]
                )
                ginv = small.tile([128, 1], f32, name="ginv", tag="ginv")
                nc.vector.reciprocal(ginv[:], gsum[:])
                gn = small.tile([128, H_DYN], f32, name="gn", tag="gn")
                nc.vector.tensor_scalar_mul(gn[:], gexp[:], ginv[:])
                m1 = small.tile([128, 1], f32, name="m1", tag="m1")
                nc.vector.reduce_max(m1[:], gn[:], axis=mybir.AxisListType.X)
                g2 = small.tile([128, H_DYN], f32, name="g2", tag="g2")
                eqm = small.tile([128, H_DYN], f32, name="eqm", tag="eqm")
                nc.vector.tensor_scalar(eqm[:], gn[:], m1[:], None, op0=ALU.is_equal)
                nc.vector.scalar_tensor_tensor(
                    g2[:], eqm[:], NEG, gn[:], op0=ALU.mult, op1=ALU.add
                )
                m2 = small.tile([128, 1], f32, name="m2", tag="m2")
                nc.vector.reduce_max(m2[:], g2[:], axis=mybir.AxisListType.X)
                sel = small.tile([128, H_DYN], f32, name="sel", tag="sel")
                nc.vector.tensor_scalar(sel[:], gn[:], m2[:], None, op0=ALU.is_ge)
                dyn = small.tile([128, H_DYN], f32, name="dyn", tag="dyn")
                nc.vector.tensor_tensor(dyn[:], gn[:], sel[:], op=ALU.mult)
                nc.tensor.matmul(
                    ps_r[:], ones_f32[:], dyn[:],
                    start=(sb == 0), stop=(sb == SB - 1),
                    skip_group_check=True,
                )
            routing_sb = small.tile([1, H], f32, name="routing_sb", tag="routing_sb")
            nc.vector.memset(routing_sb[0:1, 0:H_SH], 1.0)
            nc.scalar.mul(routing_sb[0:1, H_SH:H], ps_r[0:1, :], 1.0 / S)
            routing_bc = small.tile([128, H], f32, name="routing_bc", tag="routing_bc")
            nc.gpsimd.partition_broadcast(routing_bc[:], routing_sb[:])

            # ---- attention --------------------------------------------
            ctxT = act.tile([128, DT, S], bf16, name="ctxT", tag="ctxT")
            for h in range(H):
                ph = (h % 2) * 64        # partition base of this head
                th = h // 2              # d' tile of this head
                stexp = {}
                for kb in range(SB):
                    if qs[kb] is None:
                        continue
                    q0 = qs[kb]
                    n = S - q0
                    ps_st = psum.tile([128, S], f32, name="ps_st", tag="mm", bufs=3)
                    nc.tensor.matmul(
                        ps_st[:, 0:n],
                        kpT[ph:ph + 64, th, kb * 128:(kb + 1) * 128],
                        qpT[ph:ph + 64, th, q0:S],
                        start=True, stop=True,
                    )
                    for qb in range(q0 // 128, SB):
                        mi = mixed.get((qb, kb))
                        if mi is not None and mi >= 0:
                            sl = ps_st[:, qb * 128 - q0:(qb + 1) * 128 - q0]
                            nc.vector.tensor_tensor(
                                sl, sl, mask_tiles[mi][:], op=ALU.add
                            )
                    se = small.tile([128, S], bf16, name="stexp", tag="stexp", bufs=4)
                    nc.scalar.activation(
                        se[:, 0:n], ps_st[:, 0:n], AF.Exp, scale=1.0 / np.sqrt(DK)
                    )
                    for qb in range(q0 // 128, SB):
                        if mixed.get((qb, kb)) == -1:
                            nc.vector.memset(
                                se[:, qb * 128 - q0:(qb + 1) * 128 - q0], 0.0
                            )
                    if q0 == 0:
                        nc.vector.memset(se[:, 0:1], 1.0)
                    stexp[kb] = (se, q0, n)

                ps_ctx = psum.tile([DK + 1, S], f32, name="ps_ctx", tag="ctx", bufs=2)
                mms = []
                for kb in range(SB):
                    if kb in stexp:
                        se, q0, n = stexp[kb]
                        mms.append((vp[:, kb, h, :], se[:, 0:n], ps_ctx[:, q0:S]))
                    if qs[kb] != 0:
                        mms.append((vp[:, kb, h, :], ones_bf[:], ps_ctx[:, 0:1]))
                for i, (lhsT, rhs, dst) in enumerate(mms):
                    nc.tensor.matmul(
                        dst, lhsT, rhs,
                        start=(i == 0), stop=(i == len(mms) - 1),
                        skip_group_check=True,
                    )

                recip = small.tile([1, S], f32, name="recip", tag="recip")
                nc.vector.reciprocal(recip[:], ps_ctx[DK:DK + 1, :])
                bc = small.tile([64, S], f32, name="bc", tag="bc", bufs=2)
                nc.gpsimd.partition_broadcast(bc[:], recip[:], channels=64)
                nc.vector.scalar_tensor_tensor(
                    ctxT[ph:ph + 64, th, :],
                    ps_ctx[0:DK, :],
                    routing_bc[0:64, h:h + 1],
                    bc[:],
                    op0=ALU.mult, op1=ALU.mult,
                )

            # ---- output projection ------------------------------------
            for sb in range(SB):
                for c in range(2):
                    ps = psum.tile([128, S], f32, name="mmo_ps", tag="mm", bufs=3)
                    for t in range(DT):
                        nc.tensor.matmul(
                            ps[:],
                            ctxT[:, t, sb * 128:(sb + 1) * 128],
                            w_tiles["wo"][t][:, c * 512:(c + 1) * 512],
                            start=(t == 0),
                            stop=(t == DT - 1),
                        )
                    ob = small.tile([128, S], f32, name="ob", tag="ob", bufs=2)
                    if has_bvo:
                        nc.vector.scalar_tensor_tensor(
                            ob[:], ps[:], 1.0, bob_sb[:, c * 512:(c + 1) * 512],
                            op0=ALU.mult, op1=ALU.add,
                        )
                    else:
                        nc.scalar.copy(ob[:], ps[:])
                    nc.sync.dma_start(
                        out[b, sb * 128:(sb + 1) * 128, c * 512:(c + 1) * 512],
                        ob[:],
                    )

    nc.compile()
    return nc


def kernel(**inputs):
    q = np.asarray(inputs["q"])
    k = np.asarray(inputs["k"])
    v = np.asarray(inputs["v"])
    mask = np.asarray(inputs["mask"]).reshape(S, S)
    Wq, bq = np.asarray(inputs["Wq"]), np.asarray(inputs["bq"])
    Wk, bk = np.asarray(inputs["Wk"]), np.asarray(inputs["bk"])
    Wv, bv = np.asarray(inputs["Wv"]), np.asarray(inputs["bv"])
    Wg = np.asarray(inputs["Wg"])
    Wo, bo = np.asarray(inputs["Wo"]), np.asarray(inputs["bo"])

    bf = ml_dtypes.bfloat16
    qs, mixed, uniq = _classify_mask(mask)
    mask_key = mask.tobytes()
    has_bvo = bool(np.any(bv) or np.any(bo))
    cache_key = ("v1", mask_key, has_bvo)
    if cache_key not in _CACHE:
        _CACHE[cache_key] = _build(mask_key, qs, mixed, len(uniq), has_bvo=has_bvo)
    nc = _CACHE[cache_key]

    qT = np.ascontiguousarray(q.astype(bf).transpose(0, 2, 1))
    kT = np.ascontiguousarray(k.astype(bf).transpose(0, 2, 1))
    vT = np.ascontiguousarray(v.astype(bf).transpose(0, 2, 1))

    shared = {
        "wq": Wq.astype(bf), "wk": Wk.astype(bf), "wv": Wv.astype(bf),
        "wo": Wo.astype(bf), "wg": Wg.astype(bf),
        "bqt": np.ascontiguousarray(
            bq.astype(np.float32).reshape(DT, 128).T),
        "bkt": np.ascontiguousarray(
            bk.astype(np.float32).reshape(DT, 128).T),
    }
    if has_bvo:
        shared["bvb"] = bv.astype(np.float32).reshape(1, D)
        shared["bob"] = bo.astype(np.float32).reshape(1, D)
    if uniq:
        shared["maskt"] = np.stack(uniq, axis=0)

    in_maps = []
    for c in range(N_CORES):
        sl = slice(c * B_LOC, (c + 1) * B_LOC)
        m = dict(shared)
        m["qT"] = qT[sl]
        m["kT"] = kT[sl]
        m["vT"] = vT[sl]
        in_maps.append(m)

    from concourse.bass_utils import run_bass_kernel_spmd

    kw = {}
    if PROFILE:
        import tempfile
        kw = dict(trace=True, tmpdir=tempfile.mkdtemp(prefix="moh_trace_"))
    res = run_bass_kernel_spmd(nc, in_maps, core_ids=list(range(N_CORES)), **kw)
    LAST["exec_time_ns"] = res.exec_time_ns
    LAST["profile_json"] = res.profile_json
    if PROFILE:
        LAST["tmpdir"] = kw.get("tmpdir")
    outs = [res.results[c]["out"] for c in range(N_CORES)]
    return np.concatenate(outs, axis=0).astype(np.float32)


# revision 5
# speedup vs baseline: 3.7962x; 3.7962x over previous
"""MoH (Mixture-of-Heads) attention kernel for Trainium2, 8 NeuronCores.

Strategy: data-parallel over batch (32 batches -> 4 per core), weights
replicated, no collectives. Inside each core everything is computed with
bf16 matmuls (fp32 PSUM accumulation):

  - host pre-transposes q/k/v to [B, D, S] and casts to bf16, so the
    projections need no on-device transposes:
       qpT[d',s] = sum_d Wq[d,d'] qT[d,s]   (lhsT = Wq as stored)
       vp[s,d']  = sum_d vT[d,s] Wv[d,d']   (lhsT = vT)
  - heads live in the partition dim of qpT/kpT (64 rows each), so
    transposed scores ST[k,q] = kh @ qh^T come straight from matmuls of
    qpT/kpT slices; softmax runs without max-subtraction (scores are
    O(1)), masked blocks are skipped entirely, the causal diagonal gets
    an additive -1e9 mask tile, and the reference's "row 0 := 0" rule
    becomes "STexp[:,0] := 1" plus ones-matmuls for skipped k-blocks.
  - vp carries a ones-column so the attention matmul also produces the
    softmax denominator (row 64 of the [65, q] PSUM tile).
  - routing: gates = softmax(qpT^T @ Wg), hard top-2 of 12 via two
    reduce_max passes, mean over S via a ones-vector matmul; the
    per-(batch,head) routing scalar and 1/denominator are fused into the
    single DVE op that moves ctx^T from PSUM to SBUF.
  - out = ctxT^T @ Wo from the same partition-sliced ctxT tiles.
"""

import sys

_TRN_REPO = "/opt/trn_rl_repo"
if _TRN_REPO not in sys.path:
    sys.path.insert(0, _TRN_REPO)

import numpy as np
import ml_dtypes

B, S, D = 32, 512, 1024
H, DK = 16, 64
H_SH, K_SEL = 4, 2
H_DYN = H - H_SH
N_CORES = 8
B_LOC = B // N_CORES
SB = S // 128      # 4 s-blocks
DT = D // 128      # 8 d-tiles
NEG = -1e9

_CACHE = {}
PROFILE = False          # set by test harness to capture an NTFF trace
LAST = {}                # exec_time_ns / profile path from the last run


def _classify_mask(mask):
    """Host-side: derive block structure from the [S,S] 0/1 mask.

    Returns (qs, mixed, uniq_tiles) where
      qs[kb]    = first q (multiple of 128) kept for k-block kb, or None
      mixed[(qb,kb)] = index into uniq_tiles for blocks needing an
                  additive mask tile (maskT layout [k_local, q_local])
      uniq_tiles = list of [128,128] f32 additive tiles
    """
    m = mask.astype(bool)
    if not m[1:].any(axis=1).all():
        raise NotImplementedError(
            "a query row (>0) is fully masked; uniform-softmax fallback "
            "for fully-masked rows is not implemented"
        )
    qs = []
    mixed = {}
    uniq = []
    uniq_key = {}
    for kb in range(SB):
        first = None
        for qb in range(SB):
            blk = m[qb * 128:(qb + 1) * 128, kb * 128:(kb + 1) * 128]
            if blk.any():
                if first is None:
                    first = qb * 128
                if not blk.all():
                    add = np.where(blk.T, 0.0, np.float32(NEG)).astype(np.float32)
                    key = add.tobytes()
                    if key not in uniq_key:
                        uniq_key[key] = len(uniq)
                        uniq.append(add)
                    mixed[(qb, kb)] = uniq_key[key]
            elif first is not None:
                # fully-masked block inside the computed q range: the
                # scores are computed but must not contribute
                mixed[(qb, kb)] = -1  # sentinel: zero it after exp
        qs.append(first)
    return qs, mixed, uniq


def _build(mask_key, qs, mixed, uniq_n, b_loc=B_LOC, has_bvo=False, repeat=1):
    import concourse.bacc as bacc
    import concourse.tile as tile
    import concourse.mybir as mybir
    from contextlib import ExitStack

    f32 = mybir.dt.float32
    bf16 = mybir.dt.bfloat16
    AF = mybir.ActivationFunctionType
    ALU = mybir.AluOpType

    nc = bacc.Bacc(trn_type="TRN2", target_bir_lowering=False, debug=False)

    qT = nc.dram_tensor("qT", [b_loc, D, S], bf16, kind="ExternalInput").ap()
    kT = nc.dram_tensor("kT", [b_loc, D, S], bf16, kind="ExternalInput").ap()
    vT = nc.dram_tensor("vT", [b_loc, D, S], bf16, kind="ExternalInput").ap()
    wq = nc.dram_tensor("wq", [D, D], bf16, kind="ExternalInput").ap()
    wk = nc.dram_tensor("wk", [D, D], bf16, kind="ExternalInput").ap()
    wv = nc.dram_tensor("wv", [D, D], bf16, kind="ExternalInput").ap()
    wo = nc.dram_tensor("wo", [D, D], bf16, kind="ExternalInput").ap()
    wg = nc.dram_tensor("wg", [D, H_DYN], bf16, kind="ExternalInput").ap()
    bqt = nc.dram_tensor("bqt", [128, DT], f32, kind="ExternalInput").ap()
    bkt = nc.dram_tensor("bkt", [128, DT], f32, kind="ExternalInput").ap()
    if has_bvo:
        bvb = nc.dram_tensor("bvb", [1, D], f32, kind="ExternalInput").ap()
        bob = nc.dram_tensor("bob", [1, D], f32, kind="ExternalInput").ap()
    if uniq_n:
        maskt = nc.dram_tensor(
            "maskt", [uniq_n, 128, 128], f32, kind="ExternalInput"
        ).ap()
    out = nc.dram_tensor("out", [b_loc, S, D], f32, kind="ExternalOutput").ap()

    with tile.TileContext(nc) as tc, ExitStack() as ctx:
        const = ctx.enter_context(tc.tile_pool(name="const", bufs=1))
        act = ctx.enter_context(tc.tile_pool(name="act", bufs=2))
        small = ctx.enter_context(tc.tile_pool(name="small", bufs=2))
        psum = ctx.enter_context(tc.tile_pool(name="psum", bufs=1, space="PSUM"))

        # ---- constants -------------------------------------------------
        w_tiles = {}
        for wname, wap in (("wq", wq), ("wk", wk), ("wv", wv), ("wo", wo)):
            tl = []
            for d in range(DT):
                t = const.tile([128, D], bf16, name=f"{wname}{d}", tag=f"{wname}{d}")
                nc.sync.dma_start(t[:], wap[d * 128:(d + 1) * 128, :])
                tl.append(t)
            w_tiles[wname] = tl
        wg_tiles = []
        for d in range(DT):
            t = const.tile([128, H_DYN], bf16, name=f"wg{d}", tag=f"wg{d}")
            nc.sync.dma_start(t[:], wg[d * 128:(d + 1) * 128, :])
            wg_tiles.append(t)
        bq_sb = const.tile([128, DT], f32, name="bq_sb", tag="bq_sb")
        nc.sync.dma_start(bq_sb[:], bqt[:])
        bk_sb = const.tile([128, DT], f32, name="bk_sb", tag="bk_sb")
        nc.sync.dma_start(bk_sb[:], bkt[:])
        if has_bvo:
            bv_sb = const.tile([1, D], f32, name="bv_sb", tag="bv_sb")
            nc.sync.dma_start(bv_sb[:], bvb[:])
            bo_sb = const.tile([1, D], f32, name="bo_sb", tag="bo_sb")
            nc.sync.dma_start(bo_sb[:], bob[:])
            bvb_sb = const.tile([128, D], f32, name="bvb_sb", tag="bvb_sb")
            nc.gpsimd.partition_broadcast(bvb_sb[:], bv_sb[:])
            bob_sb = const.tile([128, D], f32, name="bob_sb", tag="bob_sb")
            nc.gpsimd.partition_broadcast(bob_sb[:], bo_sb[:])

        mask_tiles = []
        for u in range(uniq_n):
            t = const.tile([128, 128], f32, name=f"mask{u}", tag=f"mask{u}")
            nc.sync.dma_start(t[:], maskt[u])
            mask_tiles.append(t)

        ones_bf = const.tile([128, 1], bf16, name="ones_bf", tag="ones_bf")
        nc.vector.memset(ones_bf[:], 1.0)
        ones_f32 = const.tile([128, 1], f32, name="ones_f32", tag="ones_f32")
        nc.vector.memset(ones_f32[:], 1.0)

        # ---- per-batch pipeline ---------------------------------------
        for b in [bb for _ in range(repeat) for bb in range(b_loc)]:
            # input tiles
            ins = {}
            for nm, ap in (("q", qT), ("k", kT), ("v", vT)):
                t = act.tile([128, DT, S], bf16, name=f"in_{nm}", tag=f"in_{nm}", bufs=1)
                for d in range(DT):
                    nc.sync.dma_start(t[:, d, :], ap[b, d * 128:(d + 1) * 128, :])
                ins[nm] = t

            # ---- projections ------------------------------------------
            qpT = act.tile([128, DT, S], bf16, name="qpT", tag="qpT")
            kpT = act.tile([128, DT, S], bf16, name="kpT", tag="kpT")
            for dst, src, wn, bias in (
                (qpT, ins["q"], "wq", bq_sb),
                (kpT, ins["k"], "wk", bk_sb),
            ):
                for t in range(DT):
                    ps = psum.tile([128, S], f32, name="mm_ps", tag="mm", bufs=3)
                    for d in range(DT):
                        nc.tensor.matmul(
                            ps[:],
                            w_tiles[wn][d][:, t * 128:(t + 1) * 128],
                            src[:, d, :],
                            start=(d == 0),
                            stop=(d == DT - 1),
                        )
                    nc.scalar.activation(
                        dst[:, t, :], ps[:], AF.Identity,
                        bias=bias[:, t:t + 1],
                    )

            # vp: [s, h, 65] with ones column (65th) for the denominator
            vp = act.tile([128, SB, H, DK + 1], bf16, name="vp", tag="vp")
            nc.vector.memset(vp[:, :, :, DK:DK + 1], 1.0)
            for sb in range(SB):
                for c in range(2):  # d' chunks of 512
                    ps = psum.tile([128, S], f32, name="mmv_ps", tag="mm", bufs=3)
                    for d in range(DT):
                        nc.tensor.matmul(
                            ps[:],
                            ins["v"][:, d, sb * 128:(sb + 1) * 128],
                            w_tiles["wv"][d][:, c * 512:(c + 1) * 512],
                            start=(d == 0),
                            stop=(d == DT - 1),
                        )
                    src = ps[:].rearrange("p (h e) -> p h e", e=DK)
                    dst = vp[:, sb, c * 8:(c + 1) * 8, 0:DK]
                    if has_bvo:
                        nc.vector.scalar_tensor_tensor(
                            dst, src, 1.0,
                            bvb_sb[:, c * 512:(c + 1) * 512].rearrange(
                                "p (h e) -> p h e", e=DK),
                            op0=ALU.mult, op1=ALU.add,
                        )
                    else:
                        nc.vector.tensor_copy(dst, src)

            # ---- routing gates ----------------------------------------
            ps_r = psum.tile([1, H_DYN], f32, name="ps_r", tag="rsum", bufs=1)
            for sb in range(SB):
                ps_g = psum.tile([128, H_DYN], f32, name="ps_g", tag="gat", bufs=2)
                for t in range(DT):
                    nc.tensor.matmul(
                        ps_g[:],
                        qpT[:, t, sb * 128:(sb + 1) * 128],
                        wg_tiles[t][:],
                        start=(t == 0),
                        stop=(t == DT - 1),
                    )
                gexp = small.tile([128, H_DYN], f32, name="gexp", tag="gexp")
                gsum = small.tile([128, 1], f32, name="gsum", tag="gsum")
                nc.scalar.activation(
                    gexp[:], ps_g[:], AF.Exp, accum_out=gsum[:]
                )
                ginv = small.tile([128, 1], f32, name="ginv", tag="ginv")
                nc.vector.reciprocal(ginv[:], gsum[:])
                gn = small.tile([128, H_DYN], f32, name="gn", tag="gn")
                nc.vector.tensor_scalar_mul(gn[:], gexp[:], ginv[:])
                m1 = small.tile([128, 1], f32, name="m1", tag="m1")
                nc.vector.reduce_max(m1[:], gn[:], axis=mybir.AxisListType.X)
                g2 = small.tile([128, H_DYN], f32, name="g2", tag="g2")
                eqm = small.tile([128, H_DYN], f32, name="eqm", tag="eqm")
                nc.vector.tensor_scalar(eqm[:], gn[:], m1[:], None, op0=ALU.is_equal)
                nc.vector.scalar_tensor_tensor(
                    g2[:], eqm[:], NEG, gn[:], op0=ALU.mult, op1=ALU.add
                )
                m2 = small.tile([128, 1], f32, name="m2", tag="m2")
                nc.vector.reduce_max(m2[:], g2[:], axis=mybir.AxisListType.X)
                sel = small.tile([128, H_DYN], f32, name="sel", tag="sel")
                nc.vector.tensor_scalar(sel[:], gn[:], m2[:], None, op0=ALU.is_ge)
                dyn = small.tile([128, H_DYN], f32, name="dyn", tag="dyn")
                nc.vector.tensor_tensor(dyn[:], gn[:], sel[:], op=ALU.mult)
                nc.tensor.matmul(
                    ps_r[:], ones_f32[:], dyn[:],
                    start=(sb == 0), stop=(sb == SB - 1),
                    skip_group_check=True,
                )
            routing_sb = small.tile([1, H], f32, name="routing_sb", tag="routing_sb")
            nc.vector.memset(routing_sb[0:1, 0:H_SH], 1.0)
            nc.scalar.mul(routing_sb[0:1, H_SH:H], ps_r[0:1, :], 1.0 / S)
            routing_bc = small.tile([128, H], f32, name="routing_bc", tag="routing_bc")
            nc.gpsimd.partition_broadcast(routing_bc[:], routing_sb[:])

            # ---- attention --------------------------------------------
            ctxT = act.tile([128, DT, S], bf16, name="ctxT", tag="ctxT")
            for h in range(H):
                ph = (h % 2) * 64        # partition base of this head
                th = h // 2              # d' tile of this head
                stexp = {}
                for kb in range(SB):
                    if qs[kb] is None:
                        continue
                    q0 = qs[kb]
                    n = S - q0
                    ps_st = psum.tile([128, S], f32, name="ps_st", tag="mm", bufs=3)
                    nc.tensor.matmul(
                        ps_st[:, 0:n],
                        kpT[ph:ph + 64, th, kb * 128:(kb + 1) * 128],
                        qpT[ph:ph + 64, th, q0:S],
                        start=True, stop=True,
                    )
                    for qb in range(q0 // 128, SB):
                        mi = mixed.get((qb, kb))
                        if mi is not None and mi >= 0:
                            sl = ps_st[:, qb * 128 - q0:(qb + 1) * 128 - q0]
                            nc.vector.tensor_tensor(
                                sl, sl, mask_tiles[mi][:], op=ALU.add
                            )
                    se = small.tile([128, S], bf16, name="stexp", tag="stexp", bufs=4)
                    nc.scalar.activation(
                        se[:, 0:n], ps_st[:, 0:n], AF.Exp, scale=1.0 / np.sqrt(DK)
                    )
                    for qb in range(q0 // 128, SB):
                        if mixed.get((qb, kb)) == -1:
                            nc.vector.memset(
                                se[:, qb * 128 - q0:(qb + 1) * 128 - q0], 0.0
                            )
                    if q0 == 0:
                        nc.vector.memset(se[:, 0:1], 1.0)
                    stexp[kb] = (se, q0, n)

                ps_ctx = psum.tile([DK + 1, S], f32, name="ps_ctx", tag="ctx", bufs=2)
                mms = []
                for kb in range(SB):
                    if kb in stexp:
                        se, q0, n = stexp[kb]
                        mms.append((vp[:, kb, h, :], se[:, 0:n], ps_ctx[:, q0:S]))
                    if qs[kb] != 0:
                        mms.append((vp[:, kb, h, :], ones_bf[:], ps_ctx[:, 0:1]))
                for i, (lhsT, rhs, dst) in enumerate(mms):
                    nc.tensor.matmul(
                        dst, lhsT, rhs,
                        start=(i == 0), stop=(i == len(mms) - 1),
                        skip_group_check=True,
                    )

                recip = small.tile([1, S], f32, name="recip", tag="recip")
                nc.vector.reciprocal(recip[:], ps_ctx[DK:DK + 1, :])
                bc = small.tile([64, S], f32, name="bc", tag="bc", bufs=2)
                nc.gpsimd.partition_broadcast(bc[:], recip[:], channels=64)
                nc.vector.scalar_tensor_tensor(
                    ctxT[ph:ph + 64, th, :],
                    ps_ctx[0:DK, :],
                    routing_bc[0:64, h:h + 1],
                    bc[:],
                    op0=ALU.mult, op1=ALU.mult,
                )

            # ---- output projection ------------------------------------
            for sb in range(SB):
                for c in range(2):
                    ps = psum.tile([128, S], f32, name="mmo_ps", tag="mm", bufs=3)
                    for t in range(DT):
                        nc.tensor.matmul(
                            ps[:],
                            ctxT[:, t, sb * 128:(sb + 1) * 128],
                            w_tiles["wo"][t][:, c * 512:(c + 1) * 512],
                            start=(t == 0),
                            stop=(t == DT - 1),
                        )
                    ob = small.tile([128, S], f32, name="ob", tag="ob", bufs=2)
                    if has_bvo:
                        nc.vector.scalar_tensor_tensor(
                            ob[:], ps[:], 1.0, bob_sb[:, c * 512:(c + 1) * 512],
                            op0=ALU.mult, op1=ALU.add,
                        )
                    else:
                        nc.scalar.copy(ob[:], ps[:])
                    nc.sync.dma_start(
                        out[b, sb * 128:(sb + 1) * 128, c * 512:(c + 1) * 512],
                        ob[:],
                    )

    nc.compile()
    return nc


def kernel(**inputs):
    q = np.asarray(inputs["q"])
    k = np.asarray(inputs["k"])
    v = np.asarray(inputs["v"])
    mask = np.asarray(inputs["mask"]).reshape(S, S)
    Wq, bq = np.asarray(inputs["Wq"]), np.asarray(inputs["bq"])
    Wk, bk = np.asarray(inputs["Wk"]), np.asarray(inputs["bk"])
    Wv, bv = np.asarray(inputs["Wv"]), np.asarray(inputs["bv"])
    Wg = np.asarray(inputs["Wg"])
    Wo, bo = np.asarray(inputs["Wo"]), np.asarray(inputs["bo"])

    bf = ml_dtypes.bfloat16
    qs, mixed, uniq = _classify_mask(mask)
    mask_key = mask.tobytes()
    has_bvo = bool(np.any(bv) or np.any(bo))
    cache_key = ("v1", mask_key, has_bvo)
    if cache_key not in _CACHE:
        _CACHE[cache_key] = _build(mask_key, qs, mixed, len(uniq), has_bvo=has_bvo)
    nc = _CACHE[cache_key]

    qT = np.ascontiguousarray(q.astype(bf).transpose(0, 2, 1))
    kT = np.ascontiguousarray(k.astype(bf).transpose(0, 2, 1))
    vT = np.ascontiguousarray(v.astype(bf).transpose(0, 2, 1))

    shared = {
        "wq": Wq.astype(bf), "wk": Wk.astype(bf), "wv": Wv.astype(bf),
        "wo": Wo.astype(bf), "wg": Wg.astype(bf),
        "bqt": np.ascontiguousarray(
            bq.astype(np.float32).reshape(DT, 128).T),
        "bkt": np.ascontiguousarray(
            bk.astype(np.float32).reshape(DT, 128).T),
    }
    if has_bvo:
        shared["bvb"] = bv.astype(np.float32).reshape(1, D)
        shared["bob"] = bo.astype(np.float32).reshape(1, D)
    if uniq:
        shared["maskt"] = np.stack(uniq, axis=0)

    in_maps = []
    for c in range(N_CORES):
        sl = slice(c * B_LOC, (c + 1) * B_LOC)
        m = dict(shared)
        m["qT"] = qT[sl]
        m["kT"] = kT[sl]
        m["vT"] = vT[sl]
        in_maps.append(m)

    from concourse.bass_utils import run_bass_kernel_spmd

    kw = {}
    if PROFILE:
        import tempfile
        kw = dict(trace=True, tmpdir=tempfile.mkdtemp(prefix="moh_trace_"))
    res = run_bass_kernel_spmd(nc, in_maps, core_ids=list(range(N_CORES)), **kw)
    LAST["exec_time_ns"] = res.exec_time_ns
    LAST["profile_json"] = res.profile_json
    if PROFILE:
        LAST["tmpdir"] = kw.get("tmpdir")
    outs = [res.results[c]["out"] for c in range(N_CORES)]
    return np.concatenate(outs, axis=0).astype(np.float32)
